# revision 19
# baseline (speedup 1.0000x reference)
"""GAT (2-layer, 8-head) Trainium2 Bass kernel, 8-core SPMD.

Strategy:
  - Host: drop masked self-loop dups, append self-loops, sort edges by dst.
    Destination-range shard across 8 cores (12500 nodes each) -> all
    scatter-adds are core-local.  Edges packed into 128-edge tiles per
    (dst-window, src-chunk); tile order sweeps (super-window, src-chunk,
    window) so dma_gather batches stay within one int16-indexable 32768-row
    chunk of the node table.
  - Device, per layer: build a per-node table [h | al | pad] (768B rows,
    dma_gather-able) plus a core-local ar table [12500, 64] via PE matmuls
    on own node rows; AllGather the big table; stream edge tiles:
      * dma_gather h+al rows by src (int16 chunk-local idx)
      * dma_gather ar rows by (dst - core_base) from the local table
      * batched scores: s = al+ar, lrelu = max(s, .2s), ex = exp(s) (ACT)
      * selection matrix without DVE traffic: PE rank-3 outer product
        D2 = (dl - n)^2 into PSUM (host rows (dl^2, dl, 1), const rhs
        (1, -2n, n^2)), then ACT  S = exp(-50*D2)  -- exactly one-hot in
        fp32 (exp(0)=1, exp(-50)->denormal, large args underflow to 0).
      * msg = h * ex (DVE, head-broadcast AP)
      * two PE matmuls (shared stationary S): denom += S.T @ ex,
        msg_agg += S.T @ msg into per-super-window packed PSUM
        (bank-aligned 32B / 512B slices).
    Normalize AFTER aggregation (softmax without max-subtraction: exact
    rescaling; scores here are O(1) so fp32-safe), add bias, ELU.
  - Layer-2 table is built inline at layer-1 window ends; the classifier
    runs inline at layer-2 window ends. Output [12500, 40] per core,
    concatenated on host.
"""

import numpy as np

import concourse.bass as bass
import concourse.bacc as bacc
import concourse.tile as tile
from concourse import mybir
from concourse.bass_utils import run_bass_kernel_spmd
from concourse.masks import make_identity

F32 = mybir.dt.float32
I16 = mybir.dt.int16

# Problem dims (hardcoded per spec)
N = 100_000
D = 128
H = 8
C = 16
F = 128          # H * C
NCLS = 40
SLOPE = 0.2
NCORES = 8
NPC = N // NCORES          # 12500 nodes per core
P = 128
TCOL = 192                 # table cols: [h(128) | al(8) | pad(56)] -> 768B rows
ACOL = 64                  # ar table cols: [ar(8) | pad(56)] -> 256B rows
PAYC = H + F               # payload cols [ex(8) | msg(128)]
GK = 16                    # tiles per gather batch
SWW = 4                    # windows in flight (PSUM bank budget; 8/8 banks crashes)
CHUNK = 32768              # int16-indexable table rows per gather chunk


# ----------------------------------------------------------------------------
# Host-side preprocessing
# ----------------------------------------------------------------------------

def _schedule(tpwc, sww, gk):
    """tpwc: [nwin, nchunk] tiles per (window, chunk). Returns
    (batches, nwin) where batches = list of (c, [(w, first, last), ...])."""
    nwin, nch = tpwc.shape
    total = tpwc.sum(axis=1)           # tiles per window
    seen = np.zeros(nwin, np.int64)
    batches = []
    for s0 in range(0, nwin, sww):
        sw = range(s0, min(s0 + sww, nwin))
        for c in range(nch):
            run = [w for w in sw for _ in range(int(tpwc[w, c]))]
            for i in range(0, len(run), gk):
                grp = []
                for w in run[i:i + gk]:
                    first = seen[w] == 0
                    seen[w] += 1
                    last = seen[w] == total[w]
                    grp.append((w, bool(first), bool(last)))
                batches.append((c, grp))
    return batches


def _preprocess(edge_index, n, npc, ncores, chunk, sww, gk):
    src0, dst0 = edge_index[0], edge_index[1]
    keep = src0 != dst0
    loop = np.arange(n, dtype=np.int64)
    src = np.concatenate([src0[keep].astype(np.int64), loop])
    dst = np.concatenate([dst0[keep].astype(np.int64), loop])
    nch = (n + chunk - 1) // chunk
    nwin = (npc + P - 1) // P

    core = dst // npc
    nloc = dst - core * npc
    w = nloc // P
    dl = nloc - w * P
    # AllGather is done in 4 row-piece collectives; table row of node n:
    piece = npc // 4
    sr = src // npc          # owning rank of src node
    sx = src - sr * npc      # local row
    si = sx // piece
    srow = si * (piece * ncores) + sr * piece + (sx - si * piece)
    ch = srow // chunk
    key = (core * nwin + w) * nch + ch
    order = np.argsort(key, kind="stable")
    src, dst, core, w, dl, ch, key, srow = (
        a[order] for a in (src, dst, core, w, dl, ch, key, srow))

    cnt = np.bincount(key, minlength=ncores * nwin * nch).reshape(ncores, nwin, nch)
    tpwc = (cnt.max(axis=0) + P - 1) // P        # [nwin, nch]
    batches = _schedule(tpwc, sww, gk)
    nb = len(batches)

    # map (w, c, k) -> (batch, slot-in-batch)
    maxk = int(tpwc.max())
    bmap = np.full((nwin, nch, maxk), -1, np.int64)
    jmap = np.full((nwin, nch, maxk), -1, np.int64)
    kcount = np.zeros((nwin, nch), np.int64)
    for b, (c, grp) in enumerate(batches):
        for j, (wj, _, _) in enumerate(grp):
            k = kcount[wj, c]
            kcount[wj, c] += 1
            bmap[wj, c, k] = b
            jmap[wj, c, k] = j

    first_idx = np.zeros(ncores * nwin * nch + 1, np.int64)
    np.cumsum(cnt.reshape(-1), out=first_idx[1:])
    rank = np.arange(len(dst), dtype=np.int64) - first_idx[key]
    kk = rank // P
    pp = rank - kk * P

    per_core = []
    for cidx in range(ncores):
        m = core == cidx
        b_e = bmap[w[m], ch[m], kk[m]]
        j_e = jmap[w[m], ch[m], kk[m]]
        p_e = pp[m]
        pos = j_e * P + p_e                      # position within batch
        # gather idx (chunk-local src), wrapped int16 [16, gk*8] replicated
        gsrc = np.zeros((nb, gk * P), np.int64)
        gsrc[b_e, pos] = srow[m] - ch[m] * chunk
        aidx = np.zeros((nb, gk * P), np.int64)
        aidx[b_e, pos] = dst[m] - cidx * npc
        dlf = np.full((nb, gk * P), -1.0, np.float32)
        dlf[b_e, pos] = dl[m]

        # trailing invalid positions (partial batches) -> -1
        for b, (c, grp) in enumerate(batches):
            gsrc[b, len(grp) * P:] = -1
            aidx[b, len(grp) * P:] = -1

        def wrap(a):
            # position i -> [i%16, i//16], replicated to 128 partitions
            out = a.reshape(nb, gk * P // 16, 16).transpose(0, 2, 1)  # [nb,16,S]
            return np.tile(out, (1, 8, 1)).astype(np.int16)

        edl3 = np.stack([dlf * dlf, dlf, np.ones_like(dlf)], axis=1)  # [nb,3,gk*P]
        per_core.append((wrap(gsrc), wrap(aidx), edl3.astype(np.float32)))
    return tpwc, batches, per_core


def _pack_att(att_l, att_r):
    aa = np.zeros((F, 2 * H), np.float32)
    for i in range(H):
        aa[i * C:(i + 1) * C, i] = att_l[0, i, :]
        aa[i * C:(i + 1) * C, H + i] = att_r[0, i, :]
    return aa


# ----------------------------------------------------------------------------
# Device program
# ----------------------------------------------------------------------------

def _build_program(n, npc, ncores, batches, chunk, gk, sww):
    nwin = (npc + P - 1) // P
    h2 = 2 * H
    nch = (n + chunk - 1) // chunk
    nb = len(batches)

    nc = bacc.Bacc("TRN2", target_bir_lowering=False, num_devices=ncores)

    xT = nc.dram_tensor("xT", [D, npc], F32, kind="ExternalInput")
    gidx = nc.dram_tensor("gidx", [nb, P, gk * P // 16], I16, kind="ExternalInput")
    aidx = nc.dram_tensor("aidx", [nb, P, gk * P // 16], I16, kind="ExternalInput")
    edl3 = nc.dram_tensor("edl3", [nb, 3, gk * P], F32, kind="ExternalInput")
    nquad = nc.dram_tensor("nquad", [3, P], F32, kind="ExternalInput")
    w1 = nc.dram_tensor("w1", [D, F], F32, kind="ExternalInput")
    w2 = nc.dram_tensor("w2", [F, F], F32, kind="ExternalInput")
    aa1 = nc.dram_tensor("aa1", [F, h2], F32, kind="ExternalInput")
    aa2 = nc.dram_tensor("aa2", [F, h2], F32, kind="ExternalInput")
    b1r = nc.dram_tensor("b1r", [P, F], F32, kind="ExternalInput")
    b2r = nc.dram_tensor("b2r", [P, F], F32, kind="ExternalInput")
    wc = nc.dram_tensor("wc", [F, NCLS], F32, kind="ExternalInput")
    bcr = nc.dram_tensor("bcr", [P, NCLS], F32, kind="ExternalInput")
    outc = nc.dram_tensor("outc", [npc, NCLS], F32, kind="ExternalOutput")

    from contextlib import ExitStack
    with tile.TileContext(nc) as tc, ExitStack() as es:
        cpool = es.enter_context(tc.tile_pool(name="consts", bufs=1))
        dpool = es.enter_context(tc.tile_pool(name="dram", bufs=1, space="DRAM"))

        w1s = cpool.tile([D, F], F32, tag="w1s")
        w2s = cpool.tile([F, F], F32, tag="w2s")
        aa1s = cpool.tile([F, h2], F32, tag="aa1s")
        aa2s = cpool.tile([F, h2], F32, tag="aa2s")
        b1s = cpool.tile([P, F], F32, tag="b1s")
        b2s = cpool.tile([P, F], F32, tag="b2s")
        wcs = cpool.tile([F, NCLS], F32, tag="wcs")
        bcs = cpool.tile([P, NCLS], F32, tag="bcs")
        nqs = cpool.tile([3, P], F32, tag="nqs")
        for t, s in ((w1, w1s), (w2, w2s), (aa1, aa1s), (aa2, aa2s),
                     (b1r, b1s), (b2r, b2s), (wc, wcs), (bcr, bcs),
                     (nquad, nqs)):
            nc.sync.dma_start(s[:], t[:])
        ident = cpool.tile([P, P], F32, tag="ident")
        make_identity(nc, ident[:])
        zpad = cpool.tile([P, TCOL - F - H], F32, tag="zpad")
        nc.vector.memset(zpad[:], 0.0)

        # DRAM tables
        t1s = dpool.tile([npc, TCOL], F32, tag="t1s")
        t1f = dpool.tile([n, TCOL], F32, tag="t1f")
        t2s = dpool.tile([npc, TCOL], F32, tag="t2s")
        t2f = dpool.tile([n, TCOL], F32, tag="t2f")
        ar1 = dpool.tile([npc, ACOL], F32, tag="ar1")
        ar2 = dpool.tile([npc, ACOL], F32, tag="ar2")

        apool = es.enter_context(tc.tile_pool(name="phasea", bufs=2))
        appool = es.enter_context(tc.tile_pool(name="phaseap", bufs=1, space="PSUM"))

        def write_table(ts, ars, w, nw, hs, aas):
            """hs: SBUF [P, F] h rows for window w; writes h+al to ts and
            ar to local ars."""
            r0 = w * P
            nc.sync.dma_start(ts[r0:r0 + nw, 0:F], hs[:nw, :])
            htp = appool.tile([P, P], F32, tag="big")
            nc.tensor.transpose(htp[:, :nw], hs[:nw, :], ident[:nw, :nw])
            hts = apool.tile([P, P], F32, tag="hts")
            nc.vector.tensor_copy(hts[:, :nw], htp[:, :nw])
            aap = appool.tile([16, P], F32, tag="big")
            nc.tensor.matmul(aap[:, :nw], lhsT=aas[:], rhs=hts[:, :nw],
                             start=True, stop=True)
            als = apool.tile([16, P], F32, tag="als")
            nc.vector.tensor_copy(als[:, :nw], aap[:, :nw])
            altp = appool.tile([P, 16], F32, tag="big")
            nc.tensor.transpose(altp[:nw, :], als[:, :nw], ident[:16, :16])
            alts = apool.tile([P, 16], F32, tag="alts")
            nc.scalar.activation(alts[:nw, :], altp[:nw, :],
                                 mybir.ActivationFunctionType.Copy)
            nc.sync.dma_start(ts[r0:r0 + nw, F:F + H], alts[:nw, 0:H])
            nc.sync.dma_start(ts[r0:r0 + nw, F + H:TCOL], zpad[:nw, :])
            nc.sync.dma_start(ars[r0:r0 + nw, 0:H], alts[:nw, H:h2])
            nc.sync.dma_start(ars[r0:r0 + nw, H:ACOL], zpad[:nw, 0:ACOL - H])

        # ---- Phase A: layer-1 tables from x (own rows) ----
        for w in range(nwin):
            nw = min(P, npc - w * P)
            xt = apool.tile([D, P], F32, tag="xt")
            nc.sync.dma_start(xt[:, :nw], xT[:, w * P:w * P + nw])
            hp = appool.tile([P, F], F32, tag="big")
            nc.tensor.matmul(hp[:nw, :], lhsT=xt[:, :nw], rhs=w1s[:],
                             start=True, stop=True)
            hs = apool.tile([P, F], F32, tag="hs")
            nc.scalar.activation(hs[:nw, :], hp[:nw, :],
                                 mybir.ActivationFunctionType.Copy)
            write_table(t1s, ar1, w, nw, hs, aa1s)

        rg = [list(range(ncores))]
        piece = npc // 4

        def table_ag(ts, tf):
            for i in range(4):
                nc.gpsimd.collective_compute(
                    "AllGather", mybir.AluOpType.bypass, replica_groups=rg,
                    ins=[ts[i * piece:(i + 1) * piece, :]],
                    outs=[tf[i * piece * ncores:(i + 1) * piece * ncores, :]])

        table_ag(t1s, t1f)

        ep = es.enter_context(tc.tile_pool(name="edge", bufs=2))
        spool = es.enter_context(tc.tile_pool(name="sel", bufs=3))
        wpool = es.enter_context(tc.tile_pool(name="wend", bufs=2))
        d2p = es.enter_context(tc.tile_pool(name="d2psum", bufs=2, space="PSUM"))
        wp = es.enter_context(tc.tile_pool(name="wpsum", bufs=sww, space="PSUM"))

        def window_end(pw, w, layer, ts2, ars2):
            nw = min(P, npc - w * P)
            r8 = wpool.tile([P, H], F32, tag="r8")
            nc.vector.tensor_scalar(out=r8[:], in0=pw[:, 0:H],
                                    scalar1=1e-16, scalar2=None,
                                    op0=mybir.AluOpType.max)
            rr = wpool.tile([P, H], F32, tag="rr")
            nc.vector.reciprocal(rr[:], r8[:])
            rrb = bass.AP(rr[:].tensor, rr[:].offset,
                          [rr[:].ap[0], [1, H], [0, C]])
            outn = wpool.tile([P, F], F32, tag="outn")
            nc.vector.tensor_tensor(out=outn[:], in0=pw[:, H:PAYC], in1=rrb,
                                    op=mybir.AluOpType.mult)
            bs = b1s if layer == 1 else b2s
            tb = wpool.tile([P, F], F32, tag="tb")
            nc.vector.tensor_tensor(out=tb[:], in0=outn[:], in1=bs[:],
                                    op=mybir.AluOpType.add)
            te = wpool.tile([P, F], F32, tag="te")
            nc.scalar.activation(te[:], tb[:], mybir.ActivationFunctionType.Exp)
            tm = wpool.tile([P, F], F32, tag="tm")
            nc.vector.tensor_scalar(out=tm[:], in0=tb[:], scalar1=0.0,
                                    scalar2=None, op0=mybir.AluOpType.max)
            tn = wpool.tile([P, F], F32, tag="tn")
            nc.vector.tensor_scalar(out=tn[:], in0=te[:], scalar1=1.0,
                                    scalar2=0.0, op0=mybir.AluOpType.subtract,
                                    op1=mybir.AluOpType.min)
            act = wpool.tile([P, F], F32, tag="act")
            nc.vector.tensor_tensor(out=act[:], in0=tm[:], in1=tn[:],
                                    op=mybir.AluOpType.add)

            atp = appool.tile([P, P], F32, tag="big")
            nc.tensor.transpose(atp[:, :nw], act[:nw, :], ident[:nw, :nw])
            ats = wpool.tile([P, P], F32, tag="ats")
            nc.scalar.activation(ats[:, :nw], atp[:, :nw],
                                 mybir.ActivationFunctionType.Copy)
            if layer == 1:
                h2p = appool.tile([P, F], F32, tag="big")
                nc.tensor.matmul(h2p[:nw, :], lhsT=ats[:, :nw], rhs=w2s[:],
                                 start=True, stop=True)
                h2s = wpool.tile([P, F], F32, tag="h2s")
                nc.scalar.activation(h2s[:nw, :], h2p[:nw, :],
                                     mybir.ActivationFunctionType.Copy)
                write_table(ts2, ars2, w, nw, h2s, aa2s)
            else:
                clp = appool.tile([P, NCLS], F32, tag="big")
                nc.tensor.matmul(clp[:nw, :], lhsT=ats[:, :nw], rhs=wcs[:],
                                 start=True, stop=True)
                cls = wpool.tile([P, NCLS], F32, tag="cls")
                nc.vector.tensor_tensor(out=cls[:nw, :], in0=clp[:nw, :],
                                        in1=bcs[:nw, :],
                                        op=mybir.AluOpType.add)
                nc.sync.dma_start(outc[w * P:w * P + nw, :], cls[:nw, :])

        def edge_pass(tf, ars, layer, ts2, ars2):
            pwins = {}
            for b, (c, grp) in enumerate(batches):
                bt = len(grp)
                ni = bt * P
                git = ep.tile([P, gk * P // 16], I16, tag="git")
                nc.sync.dma_start(git[:], gidx[b])
                ait = ep.tile([P, gk * P // 16], I16, tag="ait")
                nc.sync.dma_start(ait[:], aidx[b])
                e3t = ep.tile([3, gk * P], F32, tag="e3t")
                nc.sync.dma_start(e3t[:], edl3[b])

                hsg = ep.tile([P, gk, TCOL], F32, tag="hsg")
                c0 = c * chunk
                cs = min(chunk, n - c0)
                nc.gpsimd.dma_gather(
                    out_ap=hsg[:, :bt, :], in_ap=tf[c0:c0 + cs, :],
                    idxs_ap=git[:, :ni // 16], num_idxs=ni, num_idxs_reg=ni,
                    elem_size=TCOL, single_packet=False)
                arg = ep.tile([P, gk, ACOL], F32, tag="arg")
                nc.gpsimd.dma_gather(
                    out_ap=arg[:, :bt, :], in_ap=ars[:, :],
                    idxs_ap=ait[:, :ni // 16], num_idxs=ni, num_idxs_reg=ni,
                    elem_size=ACOL, single_packet=False)

                # batched scores
                sc = ep.tile([P, gk, H], F32, tag="sc")
                nc.vector.tensor_tensor(
                    out=sc[:, :bt, :], in0=hsg[:, :bt, F:F + H],
                    in1=arg[:, :bt, 0:H], op=mybir.AluOpType.add)
                sca = ep.tile([P, gk, H], F32, tag="sca")
                nc.vector.tensor_scalar(out=sca[:, :bt, :], in0=sc[:, :bt, :],
                                        scalar1=SLOPE, scalar2=None,
                                        op0=mybir.AluOpType.mult)
                scl = ep.tile([P, gk, H], F32, tag="scl")
                nc.vector.tensor_tensor(out=scl[:, :bt, :], in0=sc[:, :bt, :],
                                        in1=sca[:, :bt, :],
                                        op=mybir.AluOpType.max)
                pay = ep.tile([P, gk, PAYC], F32, tag="pay")
                nc.scalar.activation(pay[:, :bt, 0:H], scl[:, :bt, :],
                                     mybir.ActivationFunctionType.Exp)

                for j, (w, first, last) in enumerate(grp):
                    if first:
                        pwins[w] = wp.tile([P, PAYC], F32, tag="pwin", name=f"pwin{w}")
                    pw = pwins[w]
                    d2 = d2p.tile([P, P], F32, tag="d2")
                    nc.tensor.matmul(d2[:], lhsT=e3t[:, j * P:(j + 1) * P],
                                     rhs=nqs[:], start=True, stop=True)
                    sel = spool.tile([P, P], F32, tag="sel")
                    nc.scalar.activation(sel[:], d2[:],
                                         mybir.ActivationFunctionType.Exp,
                                         scale=-50.0)
                    exb = bass.AP(pay[:].tensor, pay[:, j, 0:H].offset,
                                  [pay[:].ap[0], [1, H], [0, C]])
                    nc.vector.tensor_tensor(out=pay[:, j, H:PAYC],
                                            in0=hsg[:, j, 0:F], in1=exb,
                                            op=mybir.AluOpType.mult)
                    nc.tensor.matmul(pw[:], lhsT=sel[:], rhs=pay[:, j, :],
                                     start=first, stop=last)
                    if last:
                        window_end(pw, w, layer, ts2, ars2)
                        del pwins[w]

        edge_pass(t1f, ar1, 1, t2s, ar2)
        table_ag(t2s, t2f)
        edge_pass(t2f, ar2, 2, None, None)

    nc.compile()
    return nc


# ----------------------------------------------------------------------------
# Entry point
# ----------------------------------------------------------------------------

def _make_in_maps(inputs, n, npc, ncores, chunk, sww, gk):
    tpwc, batches, per_core = _preprocess(
        np.asarray(inputs["edge_index"]), n, npc, ncores, chunk, sww, gk)
    x = np.ascontiguousarray(np.asarray(inputs["x"], np.float32))
    aa1 = _pack_att(np.asarray(inputs["att_l1"]), np.asarray(inputs["att_r1"]))
    aa2 = _pack_att(np.asarray(inputs["att_l2"]), np.asarray(inputs["att_r2"]))
    b1r = np.tile(np.asarray(inputs["b1"], np.float32), (P, 1))
    b2r = np.tile(np.asarray(inputs["b2"], np.float32), (P, 1))
    bcr = np.tile(np.asarray(inputs["bc"], np.float32), (P, 1))
    nn = np.arange(P, dtype=np.float32)
    nquad = np.stack([np.ones(P, np.float32), -2.0 * nn, nn * nn])
    in_maps = []
    for c in range(ncores):
        gsrc, aidx, edl3 = per_core[c]
        xt = np.ascontiguousarray(x[c * npc:(c + 1) * npc].T)
        in_maps.append({
            "xT": xt, "gidx": gsrc, "aidx": aidx, "edl3": edl3, "nquad": nquad,
            "w1": np.asarray(inputs["W1"], np.float32),
            "w2": np.asarray(inputs["W2"], np.float32),
            "aa1": aa1, "aa2": aa2, "b1r": b1r, "b2r": b2r,
            "wc": np.asarray(inputs["Wc"], np.float32), "bcr": bcr,
        })
    return batches, in_maps


_CACHE = {}


def kernel(**inputs) -> np.ndarray:
    batches, in_maps = _make_in_maps(inputs, N, NPC, NCORES, CHUNK, SWW, GK)
    key = tuple((c, tuple(g)) for c, g in batches)
    if key not in _CACHE:
        _CACHE[key] = _build_program(N, NPC, NCORES, batches, CHUNK, GK, SWW)
    nc = _CACHE[key]
    res = run_bass_kernel_spmd(nc, in_maps, core_ids=list(range(NCORES)))
    out = np.concatenate([res.results[c]["outc"] for c in range(NCORES)], axis=0)
    return out.astype(np.float32)
